# revision 20
# baseline (speedup 1.0000x reference)
"""Trainium2 Bass kernel for the AttentionModel (encoder + LSTM + dot-attention
+ vocab projection), SPMD across 8 NeuronCores.

Sharding: encoder/LSTM/attention replicated on every core over the full batch
(B=32 is too small to shard efficiently); the [32000, 2048] projection is
sharded over the vocab dim (4000 rows/core, padded to 4096). No collectives.

All large parameters are uploaded as bf16 from the host (halves HBM traffic
and removes on-chip f32->bf16 converts). Target embeddings are gathered with
dma_gather(transpose=True) directly into the [E, TOK] layout the LSTM wants.

LSTM uses 4-way PE column-tiling: per step, the 8 gate n-chunks are computed
as 2 psum tiles [128, 512] whose partition quadrants hold 4 chunks each
(set0 = {i, g}, set1 = {f, o}), streamed concurrently via tile_position.
"""

import os
import sys

sys.path.insert(0, "/opt/trn_rl_repo")

import numpy as np
from ml_dtypes import bfloat16

import concourse.bass as bass
import concourse.tile as tile
from concourse import bacc, mybir
from concourse.bass import ts, ds
from concourse.bass_utils import run_bass_kernel_spmd
from concourse.masks import make_identity

B, S, T, E, H = 32, 100, 64, 512, 1024
SP = 128          # padded source length
VS = VT = 32000
NCORES = 8
VSH = VT // NCORES        # 4000 vocab rows per core
VSHP = 4096               # padded to 128 multiple
TOK = B * T               # 2048 tokens, t-major: j = t*32 + b
G4 = 4 * H                # 4096 gate width
F32 = mybir.dt.float32
BF16 = mybir.dt.bfloat16
I16 = mybir.dt.int16
I32 = mybir.dt.int32

NEG = -1e30
AF = mybir.ActivationFunctionType

# n-chunk -> (set, quadrant): set0 = {i(n0,n1), g(n4,n5)}, set1 = {f, o}
N2SJ = {0: (0, 0), 1: (0, 1), 2: (1, 0), 3: (1, 1),
        4: (0, 2), 5: (0, 3), 6: (1, 2), 7: (1, 3)}
N2COL = {0: [0, 1, 4, 5], 1: [2, 3, 6, 7]}


def _wrap_idx(idx_flat: np.ndarray) -> np.ndarray:
    """Host: wrap flat indices into the [128, n/16] int16 layout dma_gather
    wants (index j at [j%16, j//16], replicated over the 8 groups of 16
    partitions)."""
    n = idx_flat.shape[0]
    assert n % 16 == 0
    w = idx_flat.astype(np.int16).reshape(n // 16, 16).T  # [16, n/16]
    return np.tile(w, (8, 1)).copy()                      # [128, n/16]


def build_nc():
    nc = bacc.Bacc("TRN2", target_bir_lowering=False, debug=False,
                   enable_asserts=False, num_devices=NCORES)

    # ---- parameters -----------------------------------------------------
    p_idx_src = nc.dram_tensor("idx_src", [128, B * SP // 16], I16, kind="ExternalInput")
    p_idx_pos = nc.dram_tensor("idx_pos", [128, B * SP // 16], I16, kind="ExternalInput")
    p_idx_tgt = nc.dram_tensor("idx_tgt", [128, TOK // 16], I16, kind="ExternalInput")
    p_emb_in = nc.dram_tensor("emb_in", [VS, E], BF16, kind="ExternalInput")
    p_emb_out = nc.dram_tensor("emb_out", [VT, E], BF16, kind="ExternalInput")
    p_pos_emb = nc.dram_tensor("pos_emb", [S, E], BF16, kind="ExternalInput")
    p_wh0T = nc.dram_tensor("wh0T", [2 * E, H], BF16, kind="ExternalInput")
    p_bh0 = nc.dram_tensor("bh0", [1, H], F32, kind="ExternalInput")
    p_wihT = nc.dram_tensor("wihT", [E, G4], BF16, kind="ExternalInput")
    p_whhT = nc.dram_tensor("whhT", [H, G4], BF16, kind="ExternalInput")
    p_bsum = nc.dram_tensor("bsum", [1, G4], BF16, kind="ExternalInput")
    p_wpT = nc.dram_tensor("wpT", [2 * E + H, VSHP], BF16, kind="ExternalInput")
    p_bpw = nc.dram_tensor("bpw", [128, VSHP // 128], F32, kind="ExternalInput")
    p_len = nc.dram_tensor("lens", [B, 1], F32, kind="ExternalInput")
    p_out = nc.dram_tensor("out", [VSHP, TOK], F32, kind="ExternalOutput")

    # ---- internal DRAM --------------------------------------------------
    d_enc = nc.dram_tensor("d_enc", [B, SP, 2 * E], BF16)
    d_encT = nc.dram_tensor("d_encT", [128, 8, B, S], BF16)
    d_negm = nc.dram_tensor("d_negm", [B, S], BF16)

    _ts = bool(int(os.environ.get("SIMTRACE", "0")))
    with tile.TileContext(nc, trace_sim=_ts) as tc, \
         tc.tile_pool(name="consts", bufs=1) as consts, \
         tc.tile_pool(name="persist", bufs=1) as persist:

        ident_f = consts.tile([128, 128], F32)
        make_identity(nc, ident_f)
        ident_b = consts.tile([128, 128], BF16)
        make_identity(nc, ident_b)

        hsT = persist.tile([128, 8, T + 1, B], BF16)
        meanTb = consts.tile([128, 8, B], BF16)
        c_fold = consts.tile([2 * B, 512], F32)

        # ================= P1: gathers, enc, encT, means =================
        with tc.tile_pool(name="encTsb", bufs=1) as encT_sb, \
             tc.tile_pool(name="p1", bufs=2) as p1, \
             tc.tile_pool(name="p1ps", bufs=4, space="PSUM") as p1ps:
            encT = encT_sb.tile([128, 8, B, S], BF16)
            meanT = encT_sb.tile([128, 8, B], F32)

            idx_src = p1.tile([128, B * SP // 16], I16, tag="idx")
            nc.sync.dma_start(idx_src[:], p_idx_src[:])
            idx_pos = p1.tile([128, B * SP // 16], I16, tag="idx")
            nc.sync.dma_start(idx_pos[:], p_idx_pos[:])

            NB_CH = 8           # gather 8 batches at a time
            for half in range(B // NB_CH):
                g_in = p1.tile([128, NB_CH, E], BF16, tag="gin")
                nc.gpsimd.dma_gather(
                    g_in[:], p_emb_in[:, :],
                    idx_src[:, ts(half, NB_CH * SP // 16)],
                    NB_CH * SP, NB_CH * SP, E)
                g_pos = p1.tile([128, NB_CH, E], BF16, tag="gpos")
                nc.gpsimd.dma_gather(
                    g_pos[:], p_pos_emb[:, :],
                    idx_pos[:, ts(half, NB_CH * SP // 16)],
                    NB_CH * SP, NB_CH * SP, E)
                for bi in range(NB_CH):
                    b = half * NB_CH + bi
                    nc.sync.dma_start(d_enc[b, :, 0:E], g_in[:, bi])
                    nc.sync.dma_start(d_enc[b, :, E:2 * E], g_pos[:, bi])
                    for kc in range(8):
                        src = g_in[:, bi] if kc < 4 else g_pos[:, bi]
                        ps = p1ps.tile([128, 128], BF16, tag="tp")
                        nc.tensor.transpose(ps[:], src[:, ts(kc % 4, 128)],
                                            ident_b)
                        nc.vector.tensor_copy(encT[:, kc, b], ps[:, 0:S])
            nc.vector.tensor_reduce(meanT[:], encT[:],
                                    mybir.AxisListType.X, mybir.AluOpType.add)
            nc.scalar.mul(meanTb[:], meanT[:], 1.0 / S)
            nc.sync.dma_start(d_encT[:], encT[:])

        # ================= P2: hidden = mean @ W_h0.T + b_h0 =============
        with tc.tile_pool(name="p2", bufs=2) as p2, \
             tc.tile_pool(name="p2ps", bufs=2, space="PSUM") as p2ps:
            h_f32 = p2.tile([B, H], F32, tag="hf32")
            w0 = []
            for kc in range(8):
                wt = p2.tile([128, H], BF16, tag=f"w0_{kc}")
                nc.sync.dma_start(wt[:], p_wh0T[ts(kc, 128)])
                w0.append(wt)
            bh = p2.tile([1, H], F32, tag="bh")
            nc.sync.dma_start(bh[:], p_bh0[:])
            bhb = p2.tile([B, H], F32, tag="bhb")
            nc.gpsimd.partition_broadcast(bhb[:], bh[:])
            for n in range(2):
                ps = p2ps.tile([B, 512], F32, tag="hid")
                for kc in range(8):
                    nc.tensor.matmul(ps[:], meanTb[:, kc], w0[kc][:, ts(n, 512)],
                                     start=(kc == 0), stop=(kc == 7))
                nc.vector.tensor_add(h_f32[:, ts(n, 512)], ps[:], bhb[:, ts(n, 512)])
            nc.vector.tensor_copy(c_fold[0:B, :], h_f32[:, 0:512])
            nc.vector.tensor_copy(c_fold[B:2 * B, :], h_f32[:, 512:1024])
            # transpose h0 into hsT slot 0
            for kc in range(8):
                ps = p2ps.tile([128, B], F32, tag="htp")
                nc.tensor.transpose(ps[:], h_f32[:, ts(kc, 128)],
                                    ident_f[0:B, 0:B])
                nc.vector.tensor_copy(hsT[:, kc, 0], ps[:])

        # ========== P3 prep + fused xg-production / LSTM loop ============
        with tc.tile_pool(name="p3w", bufs=1) as p3w, \
             tc.tile_pool(name="whh", bufs=1) as whhp, \
             tc.tile_pool(name="p4s", bufs=2) as p4s, \
             tc.tile_pool(name="p4ps", bufs=2, space="PSUM") as p4ps, \
             tc.tile_pool(name="p4pt", bufs=2, space="PSUM") as p4pt:
            xT = p3w.tile([128, 4, TOK], BF16)
            wih = p3w.tile([128, 4, G4], BF16)
            bias_bc = p3w.tile([128, G4], BF16)
            whh = whhp.tile([128, 8, G4], BF16)
            ones1 = p3w.tile([1, 32], BF16)
            nc.gpsimd.memset(ones1, 1.0)

            with tc.tile_pool(name="p3prep", bufs=1) as pp:
                for ec in range(4):
                    nc.sync.dma_start(wih[:, ec], p_wihT[ts(ec, 128)])
                for kc in range(8):
                    nc.sync.dma_start(whh[:, kc], p_whhT[ts(kc, 128)])
                idx_tgt = pp.tile([128, TOK // 16], I16, tag="idxt")
                nc.sync.dma_start(idx_tgt[:], p_idx_tgt[:])
                if int(os.environ.get("XT_TGATHER", "0")):
                    # transpose-mode gather: xT[:, ec, j] = emb_out[tgt[j]][ec*128:]
                    nc.gpsimd.dma_gather(xT[:], p_emb_out[:, :], idx_tgt[:],
                                         TOK, TOK, E, transpose=True)
                else:
                    with tc.tile_pool(name="xtp", bufs=2) as xtp, \
                         tc.tile_pool(name="xtps", bufs=2, space="PSUM") as xtps:
                        for half in range(4):
                            g_x = xtp.tile([128, 4, E], BF16, tag="gx")
                            nc.gpsimd.dma_gather(
                                g_x[:], p_emb_out[:, :],
                                idx_tgt[:, ts(half, TOK // 64)],
                                TOK // 4, TOK // 4, E)
                            for mi in range(4):
                                m = half * 4 + mi
                                for ec in range(4):
                                    ps = xtps.tile([128, 128], BF16, tag="xtp")
                                    nc.tensor.transpose(
                                        ps[:], g_x[:, mi][:, ts(ec, 128)],
                                        ident_b)
                                    nc.vector.tensor_copy(
                                        xT[:, ec, ts(m, 128)], ps[:])
                brow = pp.tile([1, G4], BF16, tag="brow")
                nc.sync.dma_start(brow[:], p_bsum[:])
                if int(os.environ.get("BIG_BCAST", "1")):
                    nc.gpsimd.partition_broadcast(bias_bc[:], brow[:])
                else:
                    for bq in range(8):
                        nc.gpsimd.partition_broadcast(
                            bias_bc[:, ts(bq, 512)], brow[:, ts(bq, 512)])

            def emit_step(t):
                acts = []
                for s_ in range(2):
                    ps = p4ps.tile([128, 512], F32, tag="gates")
                    for i in range(13):
                        for j in range(4):
                            n = N2COL[s_][j]
                            if i < 4:
                                lhsT = xT[:, i, ts(t, 32)]
                                rhs = wih[:, i, ts(n, 512)]
                            elif i == 4:
                                lhsT = ones1[:]
                                rhs = bias_bc[0:1, ts(n, 512)]
                            else:
                                kc = i - 5
                                lhsT = hsT[:, kc, t]
                                rhs = whh[:, kc, ts(n, 512)]
                            nc.tensor.matmul(
                                ps[32 * j:32 * (j + 1), :], lhsT, rhs,
                                start=(i == 0), stop=(i == 12),
                                skip_group_check=True,
                                tile_position=(0, 32 * j))
                    alo = p4s.tile([64, 512], BF16, tag=f"alo{s_}")
                    nc.scalar.activation(alo[:], ps[0:64, :], AF.Sigmoid)
                    ahi = p4s.tile([64, 512], BF16, tag=f"ahi{s_}")
                    nc.scalar.activation(ahi[:], ps[64:128, :],
                                         AF.Tanh if s_ == 0 else AF.Sigmoid)
                    acts.extend([alo, ahi])
                gi_, gg, gf, go = acts
                h_fold = p4s.tile([64, 512], F32, tag="hf")
                for q in range(4):
                    cq = slice(128 * q, 128 * (q + 1))
                    t2 = p4s.tile([64, 128], F32, tag="t2")
                    nc.vector.tensor_mul(t2[:], gi_[:, cq], gg[:, cq])
                    nc.vector.tensor_mul(c_fold[:, cq], gf[:, cq], c_fold[:, cq])
                    nc.vector.tensor_add(c_fold[:, cq], c_fold[:, cq], t2[:])
                    tc_ = p4s.tile([64, 128], F32, tag="t2")
                    nc.scalar.activation(tc_[:], c_fold[:, cq], AF.Tanh)
                    nc.vector.tensor_mul(h_fold[:, cq], go[:, cq], tc_[:])
                    for half in range(2):
                        kc = 4 * half + q
                        pst = p4pt.tile([128, B], F32, tag="htp")
                        nc.tensor.transpose(
                            pst[:], h_fold[32 * half:32 * (half + 1), cq],
                            ident_f[32 * half:32 * (half + 1),
                                    32 * half:32 * (half + 1)])
                        nc.vector.tensor_copy(hsT[:, kc, t + 1], pst[:])

            for t in range(T):
                emit_step(t)

        # ================= P5 + P6 =======================================
        with tc.tile_pool(name="p56", bufs=1) as p56:
            combT = p56.tile([128, 8, T, B], BF16)

            with tc.tile_pool(name="p5c", bufs=1) as p5c, \
                 tc.tile_pool(name="p5", bufs=3) as p5, \
                 tc.tile_pool(name="p5ps", bufs=2, space="PSUM") as p5ps:
                # negmask from lengths: 0 where s < len, -1e30 where s >= len
                ones_col = p5c.tile([1, T], BF16)
                nc.gpsimd.memset(ones_col, 1.0)
                lens = p5c.tile([B, 1], F32)
                nc.sync.dma_start(lens[:], p_len[:])
                iota_i = p5c.tile([B, S], I32)
                nc.gpsimd.iota(iota_i[:], pattern=[[1, S]], base=0,
                               channel_multiplier=0)
                iota_f = p5c.tile([B, S], F32)
                nc.vector.tensor_copy(iota_f[:], iota_i[:])
                sg = p5c.tile([B, S], F32)
                nc.vector.tensor_scalar(sg[:], iota_f[:], lens[:], None,
                                        mybir.AluOpType.subtract)
                halfc = p5c.tile([B, 1], F32)
                nc.gpsimd.memset(halfc, 0.5)
                negc = p5c.tile([B, 1], F32)
                nc.gpsimd.memset(negc, NEG / 2)
                sg2 = p5c.tile([B, S], F32)
                nc.scalar.activation(sg2[:], sg[:], AF.Sign, bias=halfc[:])
                negm = p5c.tile([B, S], BF16)
                nc.scalar.activation(negm[:], sg2[:], AF.Identity,
                                     bias=negc[:], scale=NEG / 2)
                nc.sync.dma_start(d_negm[:], negm[:])
                negm0 = p5c.tile([1, B * S], BF16)
                nc.sync.dma_start(negm0[:],
                                  d_negm.ap().rearrange("b s -> (b s)")[None, :])

                for b in range(B):
                    encT_b = p5.tile([128, 8, S], BF16, tag="encTb")
                    nc.sync.dma_start(encT_b[:], d_encT[:, :, b])
                    enc_b = p5.tile([128, 2 * E], BF16, tag="encb5")
                    nc.sync.dma_start(enc_b[:], d_enc[b])

                    ps_e = p5ps.tile([T, S], F32, tag="eng")
                    for kc in range(8):
                        nc.tensor.matmul(ps_e[:], hsT[:, kc, 0:T, b],
                                         encT_b[:, kc],
                                         start=(kc == 0), stop=False)
                    nc.tensor.matmul(ps_e[:], ones_col[:],
                                     negm0[:, b * S:(b + 1) * S],
                                     start=False, stop=True)
                    expE = p5.tile([T, S], F32, tag="expE")
                    esum = p5.tile([T, 1], F32, tag="esum")
                    nc.scalar.activation(expE[:], ps_e[:], AF.Exp,
                                         accum_out=esum[:])
                    esc = p5.tile([T, 1], F32, tag="esc")
                    nc.scalar.mul(esc[:], esum[:], float(S))
                    erec = p5.tile([T, 1], F32, tag="erec")
                    nc.vector.reciprocal(erec[:], esc[:])
                    align = p5.tile([T, S], BF16, tag="align")
                    nc.vector.tensor_scalar(align[:], expE[:], erec[:], None,
                                            mybir.AluOpType.mult)
                    ps_at = p5ps.tile([128, T], BF16, tag="alT")
                    nc.tensor.transpose(ps_at[0:S, :], align[:],
                                        ident_b[0:T, 0:T])
                    alT = p5.tile([S, T], BF16, tag="alTs")
                    nc.vector.tensor_copy(alT[:], ps_at[0:S, :])
                    for mc in range(8):
                        ps_c = p5ps.tile([128, T], F32, tag="ctx")
                        nc.tensor.matmul(ps_c[:], enc_b[0:S, ts(mc, 128)],
                                         alT[:], start=True, stop=True)
                        nc.vector.tensor_copy(combT[:, mc, 0:T, b], ps_c[:])

            # ================= P6: projection ============================
            with tc.tile_pool(name="p6w", bufs=6) as p6w, \
                 tc.tile_pool(name="p6", bufs=3) as p6, \
                 tc.tile_pool(name="p6ps", bufs=4, space="PSUM") as p6ps:
                bpw = p6.tile([128, VSHP // 128], F32, tag="bpw")
                nc.sync.dma_start(bpw[:], p_bpw[:])
                wpT_r = p_wpT.ap().rearrange("(kc p) v -> p kc v", p=128)
                for vc in range(VSHP // 128):
                    wpb = p6w.tile([128, 16, 128], BF16, tag="wpb")
                    nc.sync.dma_start(wpb[:], wpT_r[:, :, ts(vc, 128)])
                    for nt in range(4):
                        ps_o = p6ps.tile([128, 512], F32, tag="out")
                        for kc in range(16):
                            if kc < 8:
                                rhs = combT[:, kc, ts(nt, 16), :]
                            else:
                                rhs = hsT[:, kc - 8,
                                          1 + nt * 16:1 + (nt + 1) * 16, :]
                            nc.tensor.matmul(ps_o[:], wpb[:, kc], rhs,
                                             start=(kc == 0), stop=(kc == 15))
                        o_sb = p6.tile([128, 512], F32, tag="osb")
                        nc.scalar.activation(o_sb[:], ps_o[:], AF.Identity,
                                             bias=bpw[:, vc:vc + 1])
                        nc.sync.dma_start(p_out[ts(vc, 128), ts(nt, 512)],
                                          o_sb[:])

    nc.finalize()
    return nc


_CACHED = {}
LAST_EXEC_NS = None


def prep_in_maps(inputs):
    src = np.asarray(inputs["source_sentences"]).astype(np.int64)
    lens = np.asarray(inputs["source_lengths"]).astype(np.float32).reshape(B, 1)
    pos = np.asarray(inputs["positions"]).astype(np.int64)
    tgt = np.asarray(inputs["target_sentences"]).astype(np.int64)
    emb_in = np.asarray(inputs["emb_in"], np.float32).astype(bfloat16)
    emb_out = np.asarray(inputs["emb_out"], np.float32).astype(bfloat16)
    pos_emb = np.asarray(inputs["pos_emb"], np.float32).astype(bfloat16)
    wh0T = np.ascontiguousarray(
        np.asarray(inputs["W_h0"], np.float32).T).astype(bfloat16)
    bh0 = np.asarray(inputs["b_h0"], np.float32).reshape(1, H)
    wihT = np.ascontiguousarray(
        np.asarray(inputs["W_ih"], np.float32).T).astype(bfloat16)
    whhT = np.ascontiguousarray(
        np.asarray(inputs["W_hh"], np.float32).T).astype(bfloat16)
    bsum = (np.asarray(inputs["b_ih"], np.float32)
            + np.asarray(inputs["b_hh"], np.float32)).reshape(1, G4).astype(bfloat16)
    wproj = np.asarray(inputs["W_proj"], np.float32)
    bproj = np.asarray(inputs["b_proj"], np.float32)

    # index prep: per-b 128-padded blocks (pad idx 0 -> junk rows, masked out)
    src_pad = np.zeros((B, SP), np.int64)
    src_pad[:, :S] = src
    pos_pad = np.zeros((B, SP), np.int64)
    pos_pad[:, :S] = pos
    idx_src = _wrap_idx(src_pad.reshape(-1))
    idx_pos = _wrap_idx(pos_pad.reshape(-1))
    # target tokens t-major: j = t*32 + b
    idx_tgt = _wrap_idx(tgt.T.reshape(-1))

    common = dict(
        idx_src=idx_src, idx_pos=idx_pos, idx_tgt=idx_tgt,
        emb_in=np.ascontiguousarray(emb_in),
        emb_out=np.ascontiguousarray(emb_out),
        pos_emb=np.ascontiguousarray(pos_emb),
        wh0T=wh0T, bh0=bh0, wihT=wihT, whhT=whhT, bsum=bsum,
        lens=lens,
    )
    in_maps = []
    for c in range(NCORES):
        wp = wproj[c * VSH:(c + 1) * VSH]          # [4000, 2048]
        wpT = np.zeros((2 * E + H, VSHP), bfloat16)
        wpT[:, :VSH] = wp.T.astype(bfloat16)
        bp = np.zeros((VSHP,), np.float32)
        bp[:VSH] = bproj[c * VSH:(c + 1) * VSH]
        bpw = np.ascontiguousarray(bp.reshape(VSHP // 128, 128).T)
        in_maps.append(dict(common, wpT=np.ascontiguousarray(wpT), bpw=bpw))

    return in_maps


def kernel(**inputs) -> np.ndarray:
    in_maps = prep_in_maps(inputs)
    if "nc" not in _CACHED:
        _CACHED["nc"] = build_nc()
    nc = _CACHED["nc"]
    trace = bool(int(os.environ.get("KTRACE", "0")))
    tmpdir = os.environ.get("KTRACE_DIR") or None
    res = run_bass_kernel_spmd(nc, in_maps, list(range(NCORES)),
                               trace=trace, tmpdir=tmpdir)
    global LAST_EXEC_NS
    LAST_EXEC_NS = res.exec_time_ns
    outs = []
    for c in range(NCORES):
        o = res.results[c]["out"]                  # [4096, 2048]
        o = o[:VSH].reshape(VSH, T, B).transpose(2, 1, 0)  # [B, T, 4000]
        outs.append(o)
    return np.concatenate(outs, axis=2)


if __name__ == "__main__":
    build_nc()
    print("build ok")
